# revision 28
# baseline (speedup 1.0000x reference)
"""Trainium2 Bass/Tile kernel for pre-LN causal multi-head self-attention.

Problem shapes (hardcoded): x (4, 2048, 512), 8 heads, dq=dv=64, fp32.

Sharding over 8 NeuronCores: core c handles batch n = c//2 and the 4 heads
h in [4*(c%2), 4*(c%2)+4).  Every core runs the SAME program (SPMD); all
per-core differences are carried by input values:
  - x:       the core's batch (2048, 512)
  - resid:   even cores: x[n] (residual); odd cores: broadcast bo rows.
             Each term of (residual + bo) is added exactly once per pair.
  - wqkv:    (512, 768) gamma-folded [Wq | Wk | Wv] column slices for the
             core's 4 heads
  - bcol:    (128, 4) q/k projection bias columns (beta @ W + b)
  - bv:      (256,) v-projection bias (zero-specialized when all-zero)
  - wo:      (256, 512) Wo rows for the core's 4 heads
  - pad01:   (128, 16) key-padding mask column per key tile
  - selgrid: (128, 1024) selector matrix for denominator broadcast
Host combines: out[n] = y_part[2n] + y_part[2n+1].

Dataflow (everything stays transposed; all matmuls run as float32r):
  LN:      bn_stats/bn_aggr; rstd = Exp(-0.5*Ln(var+eps)) on ScalarE so the
           whole kernel stays in one ACT table set; gamma/beta folded into
           weights/biases on the host.
  xnT:     PE 128x128 transposes -> xnT [d, s]; evacuated on ScalarE.
  q/k:     qT/kT = W^T @ xnT [128, 2048]; two heads per tile (partition
           halves) enabling row-packed (tile_position) score matmuls.
  v:       v [s, 4, 65] tiles with a ones column; padded key rows zeroed
           (exact key-padding mask: zero contribution to numerator AND
           denominator).
  scores:  sT[key, query] = kT^T @ qT, both heads into one 2-bank psum
           tile -> ONE fused exp [128, 2, 512-o] per key tile (scale=1/8
           folded in); causal masking by zeroing sub-diagonal p on GPSIMD
           (memset + affine_select), diagonal tiles only.
  PV:      out'[65, 512] += v'^T @ p accumulated in psum; row 64 collects
           softmax denominators via the ones column.
  norm:    denominator rows DMA-packed into dcoll (partition-legal bases),
           batched exact reciprocal, K=128 selector matmul broadcasts the
           recip row across partitions, one in-place DVE multiply.
  outproj: y = outTP0^T@Wo01 + h2/h3 unpaired + residual add on DVE.

Schedule: the main loop interleaves projection chunk m with attention
blocks m for both head-pairs (attention m needs only chunks <= m), keeping
the exp-bound ScalarE fed; m<=2 normalize + output projection are dripped
into block 3's attention loop; only chunk 3's normalize + outproj remain
in the tail.  One shared PSUM pool budgets exactly 8 banks:
pa(2) + s(2x2) + po0(1) + po1(1), with rb->pa and psy->s tag reuse.
"""

import numpy as np

S = 2048
D = 512
DQ = 64
H_PER_CORE = 4
N_CORES = 8
LN_EPS = 1e-5
NEG = -30.0

_PROGRAM = {}


def _install_tile_patch():
    """Workarounds for walrus/concourse skew in this container:

    1. This walrus build rejects instructions carrying more than one
       semaphore-wait command ("Too many sync wait commands"), but Tile's
       rust wait-assigner freely emits 2-3 waits per instruction.  After
       wait assignment, split excess waits onto EventSemaphore carrier
       instructions inserted just before the owner on the same engine.
    2. Tile's tail drain carries one wait per outstanding proc; split into
       one drain per proc.
    3. Tile's tail emits a gpsimd sem_clear (Pool ISA opcode 176) that this
       walrus rejects ("ISA wrong length").  The NRT preamble's sema_reset
       zeroes user semaphores at every execution, so the tail clear is
       redundant and skipped.
    """
    from bass_rust import SyncInfo as _SyncInfo

    from concourse import mybir, tile
    from concourse.vector_clock import ScopedClock, VectorClock

    if getattr(tile.TileContext, "_mha_patch", False):
        return

    MAXW = 1

    def _split_excess_waits(ordered, nc):
        for bb_name, insts in list(ordered.items()):
            out = []
            changed = False
            for inst in insts:
                si = inst.sync_info
                if si is None:
                    out.append(inst)
                    continue
                waits = list(si.on_wait)
                if len(waits) > MAXW:
                    changed = True
                    excess = waits[:-MAXW]
                    for k in range(0, len(excess), MAXW):
                        carrier = mybir.InstEventSemaphore(
                            name=f"wsplit-{nc.next_id()}"
                        )
                        carrier.engine = inst.engine
                        carrier.bass_scheduled_proc = inst.bass_scheduled_proc
                        carrier.bass_scheduled_scope = inst.bass_scheduled_scope
                        carrier.bass_scheduled_tick = inst.bass_scheduled_tick
                        carrier.sync_info = _SyncInfo(
                            on_wait=excess[k : k + MAXW], on_update=[]
                        )
                        out.append(carrier)
                    si.on_wait = waits[-MAXW:]
                    inst.sync_info = si
                out.append(inst)
            if changed:
                ordered[bb_name] = out

    _RustTileClockWait = tile.TileClockWait

    class _SplittingTileClockWait:
        def __init__(self, tc, ordered, **kw):
            self._inner = _RustTileClockWait(tc, ordered, **kw)
            self._ordered = ordered
            self._nc = tc.nc

        def __getattr__(self, k):
            return getattr(self._inner, k)

        def assign_waits(self, bb_name):
            self._inner.assign_waits(bb_name)
            _split_excess_waits(self._ordered, self._nc)

    tile.TileClockWait = _SplittingTileClockWait

    def _patched_drain_and_barrier(self, tick_clock, wait_clock):
        gvec = tick_clock.global_clock
        n = len(gvec)
        for i in range(n):
            if gvec[i] > 0:
                v = VectorClock([gvec[j] if j == i else 0 for j in range(n)])
                d = self.nc.sync.drain()
                wait_clock.add_sem_waits(d.ins, ScopedClock({None: v}))
        self.nc.all_engine_barrier()
        assert self.sems is not None
        popped = self.nc._tile_sem_poison_stack.pop()
        assert popped is self._sem_poison
        self.nc.all_engine_barrier()

    tile.TileContext._drain_and_barrier = _patched_drain_and_barrier

    # cayman has 208 KiB usable per partition; the stale 192 KiB constant
    # leaves 16 KiB on the table
    from concourse import tile_utils

    tile_utils.max_sbuf_usage = 208 * 1024
    tile.TileContext._mha_patch = True


def _build_program(bv_zero=False):
    _BV_ZERO = bv_zero
    from contextlib import ExitStack

    import concourse.bass as bass
    import concourse.tile as tile
    from concourse import mybir
    from concourse.masks import make_identity

    f32 = mybir.dt.float32
    f32r = mybir.dt.float32r
    AF = mybir.ActivationFunctionType

    nc = bass.Bass()

    x_d = nc.dram_tensor("x", [S, D], f32, kind="ExternalInput")
    resid_d = nc.dram_tensor("resid", [S, D], f32, kind="ExternalInput")
    wqkv_d = nc.dram_tensor("wqkv", [D, 768], f32r, kind="ExternalInput")
    bcol_d = nc.dram_tensor("bcol", [128, 4], f32, kind="ExternalInput")
    bv_d = nc.dram_tensor("bv", [256], f32, kind="ExternalInput")
    wo_d = nc.dram_tensor("wo", [256, D], f32r, kind="ExternalInput")
    pad_d = nc.dram_tensor("pad01", [128, 16], f32, kind="ExternalInput")
    sel_d = nc.dram_tensor("selgrid", [128, 1024], f32r, kind="ExternalInput")
    y_d = nc.dram_tensor("y", [S, D], f32, kind="ExternalOutput")

    _install_tile_patch()

    with tile.TileContext(nc) as tc, ExitStack() as ctx:
        consts = ctx.enter_context(tc.tile_pool(name="consts", bufs=1))
        big = ctx.enter_context(tc.tile_pool(name="big", bufs=1))
        # one shared PSUM pool; tag budget adds up to exactly 8 banks so all
        # phases can be in flight at once:
        #   pa(2) + sA(2) + sB(2) + poA(1) + poB(1) = 8
        # (rb reuses sA slots, psum_y reuses sB slots later in the kernel)
        ps = ctx.enter_context(tc.tile_pool(name="ps", bufs=2, space="PSUM"))
        xa_pool = ctx.enter_context(tc.tile_pool(name="xa", bufs=3))
        xn_pool = ctx.enter_context(tc.tile_pool(name="xn", bufs=6))
        st_pool = ctx.enter_context(tc.tile_pool(name="st", bufs=4))
        p_pool = ctx.enter_context(tc.tile_pool(name="pp", bufs=4))
        dt_pool = ctx.enter_context(tc.tile_pool(name="dt", bufs=3))
        r_pool = ctx.enter_context(tc.tile_pool(name="rr", bufs=3))
        y_pool = ctx.enter_context(tc.tile_pool(name="yy", bufs=3))

        # ---- constants and weights ----
        ident = consts.tile([128, 128], f32, tag="ident")
        make_identity(nc, ident)
        eps_col = consts.tile([128, 1], f32, tag="eps")
        nc.vector.memset(eps_col, LN_EPS)
        bcol = consts.tile([128, 4], f32, tag="bcol")
        nc.sync.dma_start(out=bcol, in_=bcol_d[:, :])
        pad01 = consts.tile([128, 16], f32, tag="pad01")
        nc.sync.dma_start(out=pad01, in_=pad_d[:, :])
        selgrid = consts.tile([128, 16 * 64], f32r, tag="selgrid")
        wo_pair0 = big.tile([128, D], f32r, tag="wop0", name="wop0")
        wo_h2 = big.tile([64, D], f32r, tag="woh2", name="woh2")
        wo_h3 = big.tile([64, D], f32r, tag="woh3", name="woh3")
        bv_bc = consts.tile([128, 256], f32, tag="bv")
        nc.sync.dma_start(out=bv_bc, in_=bv_d[None, :].to_broadcast([128, 256]))

        wqkv_r = wqkv_d.rearrange("(a p) j -> a p j", p=128)
        wqkv_sb = [
            big.tile([128, 768], f32r, tag=f"wqkv{db}", name=f"wqkv{db}")
            for db in range(4)
        ]

        # persistent big tensors
        xnT = [big.tile([128, S], f32r, tag=f"xnT{db}", name=f"xnT{db}") for db in range(4)]
        qT = [big.tile([128, S], f32r, tag=f"qT{hp}", name=f"qT{hp}") for hp in range(2)]
        kT = [big.tile([128, S], f32r, tag=f"kT{hp}", name=f"kT{hp}") for hp in range(2)]
        v_sb = [big.tile([128, 4, 65], f32r, tag=f"v{st}", name=f"vsb{st}") for st in range(16)]
        # normalized attention outputs, paired: outTP[hp] rows 0:64 = head
        # 2hp (written directly), rows 64:128 = head 2hp+1 (DMA'd from outTo)
        outTP = [big.tile([128, S], f32r, tag=f"outTP{hp}", name=f"outTP{hp}") for hp in range(2)]
        outTo = [big.tile([64, S], f32r, tag=f"outTo{hp}", name=f"outTo{hp}") for hp in range(2)]
        dcoll = big.tile([128, 512], f32, tag="dcoll", name="dcoll")
        dcoll_r0 = big.tile([128, 512], f32, tag="dcollr0", name="dcollr0")
        dcoll_r = big.tile([128, 512], f32r, tag="dcollr", name="dcollr")
        nc.vector.memset(dcoll_r.bitcast(f32), 1.0)

        # ==== interleaved main loop: chunk blk of LN/proj, then the two
        # head-pairs' attention m=blk blocks (which need only chunks <= blk).
        # This keeps ScalarE (exp-bound) fed while PE does projections. ====
        from collections import deque

        pending_work = deque()
        for blk in range(4):
            chv = blk
            # ---- phase A work for chunk blk ----
            xn_tiles = []
            for ss in range(4):
                stv = chv * 4 + ss
                x_t = xa_pool.tile([128, D], f32, tag="x", name="xt")
                nc.sync.dma_start(out=x_t, in_=x_d[stv * 128 : (stv + 1) * 128, :])
                stats = st_pool.tile([128, 6], f32, tag="stats", name="stats")
                nc.vector.bn_stats(out=stats, in_=x_t)
                mv = st_pool.tile([128, 2], f32, tag="mv", name="mv")
                nc.vector.bn_aggr(out=mv, in_=stats)
                lnv = st_pool.tile([128, 1], f32, tag="lnv", name="lnv")
                nc.scalar.activation(
                    out=lnv, in_=mv[:, 1:2], func=AF.Ln, bias=eps_col, scale=1.0
                )
                rstd = st_pool.tile([128, 1], f32, tag="rstd", name="rstd")
                nc.scalar.activation(
                    out=rstd, in_=lnv, func=AF.Exp, bias=0.0, scale=-0.5
                )
                xn_t = xn_pool.tile([128, D], f32, tag="xn", name="xnt")
                nc.vector.tensor_scalar(
                    out=xn_t,
                    in0=x_t,
                    scalar1=mv[:, 0:1],
                    scalar2=rstd,
                    op0=mybir.AluOpType.subtract,
                    op1=mybir.AluOpType.mult,
                )
                xn_tiles.append(xn_t)

            if chv == 0:
                for db in range(4):
                    nc.sync.dma_start(out=wqkv_sb[db], in_=wqkv_r[db])

            # transpose 4x4 128x128 blocks: xn [s,d] -> xnT [d,s]
            for db in range(4):
                ps_t = ps.tile([128, D], f32, tag="pa", name="pst")
                for ss in range(4):
                    nc.tensor.transpose(
                        out=ps_t[:, ss * 128 : (ss + 1) * 128],
                        in_=xn_tiles[ss][:, db * 128 : (db + 1) * 128],
                        identity=ident,
                    )
                nc.scalar.copy(
                    out=xnT[db][:, chv * 512 : (chv + 1) * 512], in_=ps_t
                )

            # q/k projections: 4 j-tiles (q-pair0, q-pair1, k-pair0, k-pair1)
            for jt in range(4):
                dst = qT[jt] if jt < 2 else kT[jt - 2]
                ps_qk = ps.tile([128, 512], f32, tag="pa", name="psqk")
                for db in range(4):
                    nc.tensor.matmul(
                        out=ps_qk,
                        lhsT=wqkv_sb[db][:, jt * 128 : (jt + 1) * 128],
                        rhs=xnT[db][:, chv * 512 : (chv + 1) * 512],
                        start=(db == 0),
                        stop=(db == 3),
                    )
                nc.vector.tensor_scalar_add(
                    out=dst[:, chv * 512 : (chv + 1) * 512],
                    in0=ps_qk,
                    scalar1=bcol[:, jt : jt + 1],
                )

            # v projection: [s, e] orientation with ones column + padding
            for ss in range(4):
                stv = chv * 4 + ss
                ps_v = ps.tile([128, 256], f32, tag="pa", name="psv")
                for db in range(4):
                    nc.tensor.matmul(
                        out=ps_v,
                        lhsT=xnT[db][:, stv * 128 : (stv + 1) * 128],
                        rhs=wqkv_sb[db][:, 512:768],
                        start=(db == 0),
                        stop=(db == 3),
                    )
                vt = v_sb[stv]
                nc.gpsimd.memset(vt.bitcast(f32)[:, :, 64:65], 1.0)
                if _BV_ZERO:
                    # (psum + 0) * pad in one op; the ones column is scaled
                    # by a separate tiny op
                    nc.vector.tensor_scalar_mul(
                        out=vt[:, :, 0:64],
                        in0=ps_v.rearrange("p (h e) -> p h e", h=4),
                        scalar1=pad01[:, stv : stv + 1],
                    )
                    nc.vector.tensor_scalar_mul(
                        out=vt[:, :, 64:65],
                        in0=vt[:, :, 64:65],
                        scalar1=pad01[:, stv : stv + 1],
                    )
                else:
                    nc.vector.tensor_tensor(
                        out=vt[:, :, 0:64],
                        in0=ps_v.rearrange("p (h e) -> p h e", h=4),
                        in1=bv_bc.rearrange("p (h e) -> p h e", h=4),
                        op=mybir.AluOpType.add,
                    )
                    nc.vector.tensor_scalar_mul(
                        out=vt[:, :, :],
                        in0=vt[:, :, :],
                        scalar1=pad01[:, stv : stv + 1],
                    )

            if chv == 0:
                # late-loaded weights (needed at normalization / outproj)
                nc.sync.dma_start(out=selgrid, in_=sel_d[:, :])
                nc.sync.dma_start(out=wo_pair0, in_=wo_d[0:128, :])
                nc.sync.dma_start(out=wo_h2, in_=wo_d[128:192, :])
                nc.sync.dma_start(out=wo_h3, in_=wo_d[192:256, :])

            # ---- attention m = blk for both head-pairs ----
            m = blk
            for hp in range(2):
                po = {}
                for ab in range(2):
                    po[ab] = ps.tile(
                        [65, 512], f32, tag=f"po{ab}", bufs=1, name=f"po{ab}"
                    )
                njt = 4 * m + 4  # key tiles 0 .. 4m+3
                for j in range(njt):
                    diag_o = 128 * (j - 4 * m) if j >= 4 * m else None
                    o = diag_o if diag_o is not None else 0
                    # both heads' scores in one 2-bank tile -> one fused exp
                    ps_s = ps.tile([128, 2, 512], f32, tag="s", name="pss")
                    for ab in range(2):
                        base = ab * 64
                        nc.tensor.matmul(
                            out=ps_s[:, ab, :],
                            lhsT=kT[hp][base : base + 64, j * 128 : (j + 1) * 128],
                            rhs=qT[hp][base : base + 64, m * 512 : (m + 1) * 512],
                            start=True,
                            stop=True,
                        )
                    p_t = p_pool.tile([128, 2, 512], f32r, tag="p", name="pt")
                    nc.scalar.activation(
                        out=p_t[:, :, o:512],
                        in_=ps_s[:, :, o:512],
                        func=AF.Exp,
                        bias=0.0,
                        scale=0.125,
                    )
                    if diag_o is not None:
                        if o > 0:
                            nc.gpsimd.memset(p_t.bitcast(f32)[:, :, 0:o], 0.0)
                        # zero strictly-below-diagonal: keep col-part>=0
                        nc.gpsimd.affine_select(
                            out=p_t[:, :, o : o + 128],
                            in_=p_t[:, :, o : o + 128],
                            compare_op=mybir.AluOpType.is_ge,
                            fill=0.0,
                            base=0,
                            pattern=[[0, 2], [1, 128]],
                            channel_multiplier=-1,
                        )
                    for ab in range(2):
                        h = hp * 2 + ab
                        nc.tensor.matmul(
                            out=po[ab],
                            lhsT=v_sb[j][:, h, 0:65],
                            rhs=p_t[:, ab, :],
                            start=(j == 0),
                            stop=(j == njt - 1),
                        )
                    if blk == 3 and pending_work:
                        fn, args = pending_work.popleft()
                        fn(*args)
                # evacuate raw numerators + denominator rows (denominator
                # sits at psum partition 64; engines can't shift partitions,
                # so stage at partition 64 and DMA-pack into dcoll)
                for ab in range(2):
                    dst = outTP[hp][0:64, :] if ab == 0 else outTo[hp]
                    nc.vector.tensor_copy(
                        out=dst[:, m * 512 : (m + 1) * 512],
                        in_=po[ab][0:64, :],
                    )
                    d_st = dt_pool.tile([65, 512], f32, tag="dst", name="dst")
                    nc.vector.tensor_copy(
                        out=d_st[64:65, :], in_=po[ab][64:65, :]
                    )
                    r = (6 * hp + 3 * ab + m) if m < 3 else (32 * (hp + 1) + ab)
                    nc.sync.dma_start(
                        out=dcoll[r : r + 1, :], in_=d_st[64:65, :]
                    )

                def norm_row(bk_h, mm):
                    # rb = broadcast of recip row; multiply into outTP/outTo
                    hq, abq = divmod(bk_h, 2)
                    bk = bk_h * 4 + mm
                    rb = ps.tile([64, 512], f32, tag="pa", name="rb")
                    nc.tensor.matmul(
                        out=rb,
                        lhsT=selgrid[:, bk * 64 : (bk + 1) * 64],
                        rhs=dcoll_r,
                        start=True,
                        stop=True,
                    )
                    dstq = outTP[hq][0:64, :] if abq == 0 else outTo[hq]
                    nc.vector.tensor_tensor(
                        out=dstq[:, mm * 512 : (mm + 1) * 512],
                        in0=dstq[:, mm * 512 : (mm + 1) * 512],
                        in1=rb,
                        op=mybir.AluOpType.mult,
                    )

                def move0(mm):
                    nc.sync.dma_start(
                        out=outTP[0][64:128, mm * 512 : (mm + 1) * 512],
                        in_=outTo[0][:, mm * 512 : (mm + 1) * 512],
                    )

                def outproj_tile(stv):
                    if True:
                        ps_y = ps.tile([128, 512], f32, tag="s", name="psy")
                        r_t = r_pool.tile([128, D], f32, tag="r", name="rt")
                        nc.sync.dma_start(
                            out=r_t,
                            in_=resid_d[stv * 128 : (stv + 1) * 128, :],
                        )
                        nc.tensor.matmul(
                            out=ps_y,
                            lhsT=outTP[0][:, stv * 128 : (stv + 1) * 128],
                            rhs=wo_pair0[:, :],
                            start=True,
                            stop=False,
                        )
                        nc.tensor.matmul(
                            out=ps_y,
                            lhsT=outTP[1][0:64, stv * 128 : (stv + 1) * 128],
                            rhs=wo_h2[:, :],
                            start=False,
                            stop=False,
                        )
                        nc.tensor.matmul(
                            out=ps_y,
                            lhsT=outTo[1][:, stv * 128 : (stv + 1) * 128],
                            rhs=wo_h3[:, :],
                            start=False,
                            stop=True,
                        )
                        y_t = y_pool.tile([128, D], f32, tag="y", name="yt")
                        nc.vector.tensor_tensor(
                            out=y_t, in0=ps_y, in1=r_t,
                            op=mybir.AluOpType.add,
                        )
                        nc.sync.dma_start(
                            out=y_d[stv * 128 : (stv + 1) * 128, :], in_=y_t
                        )

                if blk == 2 and hp == 1:
                    # all m<=2 denominators (both head-pairs) are in;
                    # reciprocal them now, then drip the normalize +
                    # outproj work into blk 3's attention loop (see
                    # pending_work) so it fills PE/DVE gaps there
                    nc.vector.reciprocal(
                        out=dcoll_r0[0:12, :], in_=dcoll[0:12, :]
                    )
                    nc.vector.tensor_copy(
                        out=dcoll_r[0:12, :], in_=dcoll_r0[0:12, :]
                    )
                    for mm in range(3):
                        pending_work.append((norm_row, (0, mm)))
                        pending_work.append((norm_row, (1, mm)))
                        pending_work.append((move0, (mm,)))
                        pending_work.append((norm_row, (2, mm)))
                        pending_work.append((norm_row, (3, mm)))
                        pending_work.append((outproj_tile, (mm * 4 + 0,)))
                        pending_work.append((outproj_tile, (mm * 4 + 1,)))
                        pending_work.append((outproj_tile, (mm * 4 + 2,)))
                        pending_work.append((outproj_tile, (mm * 4 + 3,)))

                if blk == 3:
                    while pending_work:
                        fn, args = pending_work.popleft()
                        fn(*args)
                    # m=3 denominators per head-pair, at legal bases 32/64
                    lo = 32 * (hp + 1)
                    nc.vector.reciprocal(
                        out=dcoll_r0[lo : lo + 2, :], in_=dcoll[lo : lo + 2, :]
                    )
                    nc.vector.tensor_copy(
                        out=dcoll_r[lo : lo + 2, :], in_=dcoll_r0[lo : lo + 2, :]
                    )
                    norm_row(hp * 2 + 0, 3)
                    norm_row(hp * 2 + 1, 3)
                    if hp == 0:
                        nc.sync.dma_start(
                            out=outTP[0][64:128, 3 * 512 : 4 * 512],
                            in_=outTo[0][:, 3 * 512 : 4 * 512],
                        )
                    else:
                        for ss in range(4):
                            outproj_tile(12 + ss)

    return nc


def _get_program(bv_zero=False):
    if bv_zero not in _PROGRAM:
        _PROGRAM[bv_zero] = _build_program(bv_zero)
    return _PROGRAM[bv_zero]


def _make_in_maps(inputs):
    x = np.ascontiguousarray(np.asarray(inputs["x"], dtype=np.float32))
    lengths = np.asarray(inputs["key_value_sequence_lengths"]).astype(np.int64)
    Wq = np.asarray(inputs["Wq"], dtype=np.float32)
    bq = np.asarray(inputs["bq"], dtype=np.float32)
    Wkv = np.asarray(inputs["Wkv"], dtype=np.float32)
    bkv = np.asarray(inputs["bkv"], dtype=np.float32)
    Wo = np.asarray(inputs["Wo"], dtype=np.float32)
    bo = np.asarray(inputs["bo"], dtype=np.float32)
    gamma = np.asarray(inputs["gamma"], dtype=np.float32)
    beta = np.asarray(inputs["beta"], dtype=np.float32)

    H = 8
    Wk = Wkv[:, : H * DQ]
    Wv = Wkv[:, H * DQ :]
    bk = bkv[: H * DQ]
    bv_full = bkv[H * DQ :]

    in_maps = []
    for c in range(N_CORES):
        n = c // 2
        h0 = 4 * (c % 2)
        hsel = slice(h0 * DQ, (h0 + 4) * DQ)  # 256 contiguous columns

        wq_s = Wq[:, hsel]
        wk_s = Wk[:, hsel]
        wv_s = Wv[:, hsel]
        wqkv = np.concatenate(
            [gamma[:, None] * wq_s, gamma[:, None] * wk_s, gamma[:, None] * wv_s],
            axis=1,
        ).astype(np.float32)
        bq_eff = beta @ wq_s + bq[hsel]
        bk_eff = beta @ wk_s + bk[hsel]
        bv_eff = beta @ wv_s + bv_full[hsel]
        bcol = np.concatenate([bq_eff, bk_eff]).reshape(4, 128).T.copy()
        wo_s = Wo[hsel, :].astype(np.float32)

        ln = int(lengths[n])
        b_idx = np.arange(128)[:, None]
        j_idx = np.arange(16)[None, :]
        pad01 = ((128 * j_idx + b_idx) < ln).astype(np.float32)

        resid = x[n] if c % 2 == 0 else np.ascontiguousarray(
            np.broadcast_to(bo, (S, D)).astype(np.float32)
        )

        sel = np.zeros((128, 1024), np.float32)
        for h in range(4):
            hp_, ab_ = divmod(h, 2)
            for m in range(4):
                blk = h * 4 + m
                if m < 3:
                    row = 6 * hp_ + 3 * ab_ + m
                else:
                    row = 32 * (hp_ + 1) + ab_
                sel[row, blk * 64 : (blk + 1) * 64] = 1.0

        in_maps.append(
            {
                "x": x[n],
                "selgrid": sel,
                "resid": resid,
                "wqkv": wqkv,
                "bcol": np.ascontiguousarray(bcol, dtype=np.float32),
                "bv": bv_eff.astype(np.float32),
                "wo": wo_s,
                "pad01": np.ascontiguousarray(pad01, dtype=np.float32),
            }
        )
    return in_maps


def kernel_run(inputs, trace=False):
    from concourse.bass_utils import run_bass_kernel_spmd

    in_maps = _make_in_maps(inputs)
    bv_zero = all(
        not np.any(np.asarray(m["bv"], dtype=np.float32)) for m in in_maps
    )
    nc = _get_program(bv_zero)
    res = run_bass_kernel_spmd(nc, in_maps, list(range(N_CORES)), trace=trace)
    parts = [res.results[c]["y"] for c in range(N_CORES)]
    out = np.stack(
        [parts[2 * n] + parts[2 * n + 1] for n in range(4)], axis=0
    ).astype(np.float32)
    return out, res


def kernel(**inputs) -> np.ndarray:
    out, _ = kernel_run(inputs)
    return out
